# revision 21
# baseline (speedup 1.0000x reference)
"""BinaryConv (BN + sign-binarize + 3x3 binary conv) on 8 Trainium2 NeuronCores.

Strategy (data-parallel over batch, per sharding hint):
  - Each of the 8 cores gets 4 of the 32 images; weights/gamma/beta replicated.
  - Per-core BN partial stats (mean, mean-square per channel) via bn_stats,
    exchanged across cores with a custom 3-round XOR AllGather-hypercube
    built on remote_dma_broadcast (direct SBUF->SBUF peer DMAs + semaphore
    waits inside a tile_critical): round r sends slots [0,2^r) to peer
    (me^2^r), landing at its slots [2^r,2^(r+1)), so slot s always holds
    core (me^s)'s payload and addressing is compile-time uniform SPMD.
    Then one 8-slot DVE reduce: sync-BN exact. This replaces the ncfw CC
    AllGather, whose one-shot cold cost measured ~55us trigger-to-data
    (rendezvous + staged triggers + mesh) vs ~10-25us for the hypercube.
    A dummy CC AllGather (result unused, pinned after the exchange so
    Tile cannot hoist it ahead of the critical section's entry clock)
    stays in the NEFF purely to get the runtime's CC gang-launch
    rendezvous — without any collective the 8 executions stagger by ~ms,
    which the remote_dma waits would eat as dead time.
  - Binarize via ScalarE Sign(gamma*x + (beta*std - mean*gamma)) — equivalent
    to the reference BN+sign since std>0 — into a zero-padded per-image
    layout (33-wide rows with one SHARED pad column per row, both ci-halves
    stacked) in fp8 e4m3.
  - 3x3 conv = 9 shifted DoubleRow fp8 matmuls (contraction 256 in one pass)
    accumulated in PSUM. +/-1 inputs with fp32 PSUM accumulation are exact
    (integer sums), so the conv matches the fp32 reference bit-for-bit.
  - Weights: sign() on ScalarE, transposed to [ci][2][o] via PE transpose.
  - Output is stored partition-major ([128][img][oblk][h][w]) so each
    per-chunk store is one contiguous 1408B burst per partition instead of
    11 separate 128B rows; the host un-permutes after the gather.
  - Post-collective path (readback -> stats math -> first binarize slab ->
    first matmul) is latency-tuned: single strided rank-reduction, fused
    variance algebra, 12-row first binarize slab, and the final image's
    PSUM copies / output DMAs split across DVE/ACT and Sync/ACT queues.
"""

import numpy as np

import concourse.tile as tile
from concourse import bacc, mybir
from concourse.bass_utils import run_bass_kernel_spmd
from concourse.masks import make_identity

F32 = mybir.dt.float32
BF16 = mybir.dt.bfloat16
FP8 = mybir.dt.float8e4

N_CORES = 8
N = 32            # full batch
NLOC = N // N_CORES  # images per core
C = 256           # channels (in == out)
HW = 32           # spatial
CB = C // 128     # ci partition blocks
OB = C // 128     # o partition blocks
EPS = 1e-5

PADW = HW + 1     # padded row width: ONE shared pad column per row — the
                  # zero at 33r+32 serves as right-pad of row r AND left-pad
                  # of row r+1, since both reads want zero
IMG_PAD = 1128    # 34 top zeros + 32*33 interior + 33+ bottom zeros, mult of 8
INT0 = 34         # interior start offset (after the 34-zero top pad)
# output row-chunks (r0, r1): each chunk's matmul free dim = (r1-r0)*33 <= 512
CHUNKS = [(0, 11), (11, 22), (22, 32)]
TAPS = [(dy, dx) for dy in range(3) for dx in range(3)]


def _build_body(ctx, nc, tc, x_d, g_d, be_d, w_d, y_d, cc_in, cc_out):
    # ---------------- pools ----------------
    const = ctx.enter_context(tc.tile_pool(name="const", bufs=1))
    xin_p = ctx.enter_context(tc.tile_pool(name="xin", bufs=1))
    wpool = ctx.enter_context(tc.tile_pool(name="wpool", bufs=1))
    apool = ctx.enter_context(tc.tile_pool(name="apool", bufs=1))
    stat_p = ctx.enter_context(tc.tile_pool(name="stat", bufs=1))
    out_p = ctx.enter_context(tc.tile_pool(name="outp", bufs=1))
    ps_tr = ctx.enter_context(tc.tile_pool(name="pstr", bufs=2, space="PSUM"))
    ps_acc = ctx.enter_context(tc.tile_pool(name="psacc", bufs=1, space="PSUM"))

    # ---------------- load x (stats-critical) ----------------
    # Alternate the two HWDGE issue engines (Sync, Scalar) to halve the
    # descriptor-generation serialization on the input path.
    xin = []
    x_dmas = []
    for b in range(CB):
        xb = xin_p.tile([128, NLOC, HW, HW], F32, name=f"xin{b}", tag=f"xin{b}")
        for i in range(NLOC):
            eng = nc.sync if (b * NLOC + i) % 2 == 0 else nc.scalar
            x_dmas.append(
                eng.dma_start(
                    out=xb[:, i], in_=x_d[i, 128 * b : 128 * (b + 1), :, :]
                )
            )
        xin.append(xb)

    # ---------------- weight prep (independent of stats) ----------------
    ident = const.tile([128, 128], BF16, name="ident")
    make_identity(nc, ident[:])


    # ---------------- zero only the padding of the activation buffers ------
    # (interior is fully overwritten by binarize; tiny strided memsets keep
    # both DVE and the collective-trigger path free)
    apad = [None] * NLOC
    for i in range(NLOC):
        ap = apool.tile([128, CB, IMG_PAD], FP8, name=f"apad{i}",
                        tag=f"apad{i}")
        nc.gpsimd.memset(ap[:, :, 0:INT0], 0.0)
        # shared pad column of each row r sits at INT0 + 33r + 32
        gaps = ap[:, :, INT0 + HW : INT0 + HW + PADW * HW].rearrange(
            "p b (h w) -> p b h w", w=PADW
        )[:, :, :, 0:1]
        nc.gpsimd.memset(gaps, 0.0)
        nc.gpsimd.memset(ap[:, :, INT0 + PADW * HW : IMG_PAD], 0.0)
        apad[i] = ap

    # warm-up rhs: 260 bf16 cols of ones (content irrelevant; the warm-up
    # chain's timing is set by an explicit dep on the stats-readback DMA)
    junk2 = stat_p.tile([128, 260], BF16, name="junk2")
    nc.gpsimd.memset(junk2[:], 1.0)

    # ---------------- local BN stats ----------------
    stats_rec = []
    for b in range(CB):
        xb = xin[b]
        rec = stat_p.tile([128, 2 * NLOC, 6], F32, name=f"rec{b}", tag=f"rec{b}")
        for i in range(NLOC):
            for h in range(2):
                nc.vector.bn_stats(
                    out=rec[:, 2 * i + h, :],
                    in_=xb[:, i, 16 * h : 16 * (h + 1), :].rearrange(
                        "p h w -> p (h w)"
                    ),
                )
        stats_rec.append(rec)

    # pack [mean_b, meansq_b] per ci-block into slot 0 of the gather buffer
    xgat = stat_p.tile([128, N_CORES, 2 * CB], F32, name="xgat", tag="xgat")
    tmp1 = stat_p.tile([128, 1], F32, name="tmp1")
    for b in range(CB):
        mv = stat_p.tile([128, 2], F32, name=f"mv{b}", tag=f"mv{b}")
        nc.vector.bn_aggr(out=mv[:], in_=stats_rec[b][:])
        nc.vector.tensor_copy(
            out=xgat[:, 0, 2 * b : 2 * b + 1], in_=mv[:, 0:1]
        )
        nc.vector.tensor_mul(tmp1[:], mv[:, 0:1], mv[:, 0:1])
        nc.vector.tensor_add(
            xgat[:, 0, 2 * b + 1 : 2 * b + 2], mv[:, 1:2], tmp1[:]
        )

    # ------ stats exchange: 3-round XOR AllGather-hypercube (remote_dma) ----
    # Replaces the ncfw CC AllGather (~55us trigger-to-data measured cold:
    # rendezvous + staged triggers + mesh) with direct SBUF->SBUF peer DMAs:
    # round r sends my slots [0, 2^r) to peer (me ^ 2^r), landing at its
    # slots [2^r, 2^(r+1)); slot s always holds core (me^s)'s payload, so
    # addressing is compile-time uniform SPMD. No adds inside the section
    # (gpsimd ucode ping-pong costs ~5us/switch); one 8-slot DVE reduce in
    # Tile-land afterwards, ordered by the section's exit barrier.
    xrecv = [nc.alloc_semaphore(f"xa_recv{r}") for r in range(3)]
    xlsem = nc.alloc_semaphore("xa_lsem")
    xpsem = nc.alloc_semaphore("xa_psem")
    gp = nc.gpsimd
    with tc.tile_critical(name="xchg"):
        for r in range(3):
            w = 1 << r
            rdests = [None] * 8
            rdests[w] = (0, w)
            gp.remote_dma_broadcast(
                out_ap=xgat[:, w : 2 * w, :],
                in_ap=xgat[:, 0:w, :],
                remote_sem=xrecv[r],
                local_sem=xlsem,
                rdests=rdests,
            ).then_inc(xpsem, 1)
        # Pool enters the section bare and runs the descgens during the
        # stats computation; this marker carries the data-dep wait (slot 0
        # packed) gating the triggers below.
        tc.wait_critical_data_deps()
        gp.wait_ge(xpsem, 3)
        for r in range(3):
            gp.trigger_dma(count=1)
            gp.wait_ge(xrecv[r], 2)
        # no end-of-exec sem clears: the runtime zeroes semaphores between
        # executions (verified: every rep re-waits), and a RANGE_CLEAR with
        # in-flight SDMA references wedges the device.
    # ------- weight prep: gated on the last x DMA so the 2.25MB W transfer
    # never queues ahead of the stats-critical x loads.
    wsign = []
    for o in range(OB):
        wraw = wpool.tile([128, C, 3, 3], F32, name=f"wraw{o}", tag=f"wraw{o}")
        # Issue-order dep only (sync=False): W's descriptors enqueue right
        # BEHIND all of x's, so x keeps full HBM bandwidth and W fills the
        # tail (done ~27us, before the stats pack). Both alternatives
        # measured worse: ungated, Tile hoists W AHEAD of x (stats +6us,
        # 129-204us total); completion-gated (sync=True), W finishes ~33-35us
        # and the critical section's entry clock sweeps that in, stalling
        # the exchange chain on all 8 cores.
        wdma = (nc.sync if o == 0 else nc.scalar).dma_start(
            out=wraw[:], in_=w_d[128 * o : 128 * (o + 1), :, :, :]
        )
        tile.add_dep_helper(
            wdma.ins, x_dmas[-1].ins, sync=False, reason="W issued after x"
        )
        ws = wpool.tile([128, C, 3, 3], BF16, name=f"wsign{o}", tag=f"wsign{o}")
        wsig_i = nc.scalar.activation(
            out=ws[:], in_=wraw[:], func=mybir.ActivationFunctionType.Sign
        )
        wsign.append(ws)

    # preload the Sqrt activation table while ScalarE is otherwise idle:
    # the real Sqrt right after the stats exchange then skips the ~1.3us
    # ACT_TABLE_LOAD that used to sit on the post-collective critical path
    sqwarm = stat_p.tile([128, 1], F32, name="sqwarm")
    nc.vector.memset(sqwarm[:], 1.0)
    sq_i = nc.scalar.activation(
        out=sqwarm[:], in_=sqwarm[:], func=mybir.ActivationFunctionType.Sqrt
    )
    tile.add_dep_helper(
        sq_i.ins, wsig_i.ins, sync=True, reason="sqrt table after W signs"
    )

    # transposed binarized weights, DoubleRow layout:
    # wT[ci_local, tap*OB+o, ci_half, o_local]  (fp8)
    wT = wpool.tile([128, len(TAPS) * OB, CB, 128], FP8, name="wT")
    for t, (dy, dx) in enumerate(TAPS):
        for b in range(CB):
            for o in range(OB):
                ptr = ps_tr.tile([128, 128], BF16, name="ptr", tag="ptr", bufs=2)
                nc.tensor.transpose(
                    ptr[:], wsign[o][:, 128 * b : 128 * (b + 1), dy, dx], ident[:]
                )
                nc.vector.tensor_copy(
                    out=wT[:, t * OB + o, b, :], in_=ptr[:]
                )

    # reduce the 8 gathered per-core payloads into global sums (ordered
    # after the exchange by the critical section's exit barrier)
    gs = stat_p.tile([128, 2 * CB], F32, name="gs")
    nc.vector.tensor_reduce(
        gs[:],
        xgat[:].rearrange("p k s -> p s k"),
        axis=mybir.AxisListType.X,
        op=mybir.AluOpType.add,
    )
    smean = gs[:].rearrange("p (b s) -> p b s", s=2)[:, :, 0]  # [128, CB]
    smsq = gs[:].rearrange("p (b s) -> p b s", s=2)[:, :, 1]

    # per-channel scale/shift computed as wide [128, CB] ops.
    # Since std > 0:  sign((x-mean)*gamma/std + beta)
    #              == sign(gamma*x + (beta*std - mean*gamma))
    # so scale = gamma (known before the AllReduce!) and
    # shift = beta*std - mean*gamma  (no reciprocal needed).
    eps_t = const.tile([128, 1], F32, name="eps_t")
    nc.vector.memset(eps_t[:], EPS)
    gam = stat_p.tile([128, CB], F32, name="gam")
    bet = stat_p.tile([128, CB], F32, name="bet")
    for b in range(CB):
        nc.sync.dma_start(out=gam[:, b : b + 1], in_=g_d[128 * b : 128 * (b + 1), :])
        nc.sync.dma_start(out=bet[:, b : b + 1], in_=be_d[128 * b : 128 * (b + 1), :])
    inv = 1.0 / N_CORES
    # msqr = inv^2 * smean^2 in one fused op; var = inv*smsq - msqr
    msqr = stat_p.tile([128, CB], F32, name="msqr")
    msqr_i = nc.vector.scalar_tensor_tensor(
        out=msqr[:],
        in0=smean,
        scalar=inv * inv,
        in1=smean,
        op0=mybir.AluOpType.mult,
        op1=mybir.AluOpType.mult,
    )
    var_t = stat_p.tile([128, CB], F32, name="var_t")
    nc.vector.scalar_tensor_tensor(
        out=var_t[:],
        in0=smsq,
        scalar=inv,
        in1=msqr[:],
        op0=mybir.AluOpType.mult,
        op1=mybir.AluOpType.subtract,
    )
    # neg_mg = -(mean)*gamma, computed on DVE in parallel with var
    neg_mg = stat_p.tile([128, CB], F32, name="neg_mg")
    nc.vector.scalar_tensor_tensor(
        out=neg_mg[:],
        in0=smean,
        scalar=-inv,
        in1=gam[:],
        op0=mybir.AluOpType.mult,
        op1=mybir.AluOpType.mult,
    )
    # sqrt on ScalarE; shift = beta*std + neg_mg as two wide DVE ops (DVE is
    # idle here; keeps the ScalarE queue clear for the binarize that follows)
    std_t = stat_p.tile([128, CB], F32, name="std_t")
    nc.scalar.activation(
        out=std_t[:],
        in_=var_t[:],
        func=mybir.ActivationFunctionType.Sqrt,
        bias=eps_t[:],
        scale=1.0,
    )
    sh_t = stat_p.tile([128, CB], F32, name="sh_t")
    nc.vector.tensor_mul(sh_t[:], std_t[:], bet[:])
    nc.vector.tensor_add(sh_t[:], sh_t[:], neg_mg[:])
    scale_t = [gam[:, b : b + 1] for b in range(CB)]
    shift_t = [sh_t[:, b : b + 1] for b in range(CB)]

    # warm-up matmuls (results discarded) — sustain ~100% PE duty through the
    # stat-math + binarize window so the HAM clock ramp COMPLETES before the
    # conv burst. Tiny 4-col warm-ups are dispatch-bound (~4% duty) and leave
    # the clock at mid state: the first ~9 conv matmuls then run at ~313ns
    # instead of ~155ns (measured). 13 x 260-col bf16 matmuls cover the
    # ~3.5us window at full duty and end right as the binarize slabs land.
    for k in range(14):
        pw = ps_tr.tile([128, 260], F32, name="pw", tag="ptr", bufs=2)
        wu = nc.tensor.matmul(pw[:], ident[:], junk2[:], start=True, stop=True)
        if k == 0:
            # start the ramp chain as soon as the exchanged sums land (msqr
            # is the first post-exchange consumer); the 14 matmuls span the
            # stat-math + first-binarize window, ending right at conv start
            tile.add_dep_helper(
                wu.ins, msqr_i.ins, sync=True, reason="warmups with stat math"
            )

    # ---------------- binarize into padded layout (fp8, DoubleRow pairs) ----
    # Row-halves so the first conv chunk (rows 0..13) can start as soon as
    # the top halves of both ci-blocks are written.
    # first slab is just the 12 rows conv chunk 0 needs, so the first
    # matmul starts ~0.9us sooner after the stats land
    BROWS = [(0, 12), (12, 32)]
    for i in range(NLOC):
        for (hr0, hr1) in BROWS:
            for b in range(CB):
                interior = apad[i][:, b, INT0 : INT0 + PADW * HW].rearrange(
                    "p (h w) -> p h w", w=PADW
                )[:, hr0:hr1, 0:HW]
                nc.scalar.activation(
                    out=interior,
                    in_=xin[b][:, i, hr0:hr1, :],
                    func=mybir.ActivationFunctionType.Sign,
                    scale=scale_t[b],
                    bias=shift_t[b],
                )

    # ---------------- conv: 9 shifted DoubleRow matmuls, PSUM accumulate ----
    # Chunk-outer / taps-inner: each row-chunk's accumulation closes ~3-7us
    # before the image's last matmul, so its PSUM copy + output store drain
    # DURING compute — the final image no longer flushes 1MB after the last
    # matmul, only its last chunk.
    for i in range(NLOC):
        osb = {}
        for o in range(OB):
            osb[o] = out_p.tile([128, HW, HW], F32, name=f"osb{o}",
                                tag=f"osb{o}", bufs=2)
        for ci, (r0, r1) in enumerate(CHUNKS):
            ncols = (r1 - r0) * PADW
            psum = {}
            for o in range(OB):
                psum[o] = ps_acc.tile(
                    [128, ncols], F32, name=f"acc{o}_{ci}",
                    tag=f"acc{o}_{ci}", bufs=1,
                )
            for t, (dy, dx) in enumerate(TAPS):
                # interior(r0+dy-1, dx-1) = INT0 + 33*(r0+dy-1) + dx-1
                off = PADW * (r0 + dy) + dx
                for o in range(OB):
                    nc.tensor.matmul(
                        psum[o][:],
                        wT[:, t * OB + o, :, :],
                        apad[i][:, :, off : off + ncols],
                        start=t == 0,
                        stop=t == len(TAPS) - 1,
                        perf_mode=mybir.MatmulPerfMode.DoubleRow,
                    )
            for o in range(OB):
                src = psum[o][:].rearrange("p (r c) -> p r c", c=PADW)[
                    :, :, 0:HW
                ]
                # o=1 copy+store ride the ACT queue once binarize is done
                # (i>=2), halving the per-chunk serial copy+issue chain;
                # the output DMA issues from the same engine as its copy.
                if i >= 2 and o == 1:
                    nc.scalar.copy(out=osb[o][:, r0:r1, :], in_=src)
                    deng = nc.scalar
                else:
                    nc.vector.tensor_copy(out=osb[o][:, r0:r1, :], in_=src)
                    deng = nc.sync
                deng.dma_start(
                    out=y_d[:, i, o, r0:r1, :],
                    in_=osb[o][:, r0:r1, :],
                )

    # ---------------- dummy collective (gang-launch rendezvous) ------------
    # A CC-NEFF makes the runtime co-schedule the 8 per-core executions
    # (cores start within ~20us; without any collective they stagger by ~ms,
    # which the remote_dma exchange would eat as dead wait time). Emitted
    # LAST so nothing gates on its ~40us cold completion — it was measured
    # to stall the critical section until mesh-end when emitted first. The
    # gathered result is never consumed; the mesh runs on the TOPSP engines
    # concurrently with the conv tail / kernel drain.
    ccdummy = stat_p.tile([128, 2 * CB], F32, name="ccdummy")
    nc.vector.memset(ccdummy[:], 0.0)
    ccb = nc.gpsimd.dma_start(out=cc_in[:, :], in_=ccdummy[:])
    cc_i = nc.gpsimd.collective_compute(
        "AllGather",
        mybir.AluOpType.bypass,
        replica_groups=[list(range(N_CORES))],
        ins=[cc_in.ap().opt()],
        outs=[cc_out.ap().opt()],
    )
    # Pin the dummy CC behind the first post-exchange instruction: left
    # unpinned, Tile hoists it to the front of gpsimd's schedule and the
    # critical section's entry clock then waits for the CC mesh (~90us).
    # Scheduled here its mesh overlaps the conv and finishes before drain.
    tile.add_dep_helper(ccb.ins, msqr_i.ins, sync=True, reason="CC after stats")
    tile.add_dep_helper(cc_i.ins, msqr_i.ins, sync=True, reason="CC after stats")


_CACHE: dict = {}


def _build():
    if "nc" in _CACHE:
        return _CACHE["nc"]
    nc = bacc.Bacc(
        "TRN2", target_bir_lowering=False, debug=False, num_devices=N_CORES
    )
    x_d = nc.dram_tensor("x", [NLOC, C, HW, HW], F32, kind="ExternalInput")
    g_d = nc.dram_tensor("gamma", [C, 1], F32, kind="ExternalInput")
    be_d = nc.dram_tensor("beta", [C, 1], F32, kind="ExternalInput")
    w_d = nc.dram_tensor("w", [C, C, 3, 3], F32, kind="ExternalInput")
    # partition-major output layout: per-partition contiguous 1408B DMA
    # writes; the host un-permutes to [NLOC, C, H, W] for free
    y_d = nc.dram_tensor("y", [128, NLOC, OB, HW, HW], F32, kind="ExternalOutput")
    cc_in = nc.dram_tensor("cc_in", [128, 2 * CB], F32)
    cc_out = nc.dram_tensor(
        "cc_out", [N_CORES, 128, 2 * CB], F32, addr_space="Shared"
    )

    from contextlib import ExitStack

    with tile.TileContext(nc) as tc, ExitStack() as ctx:
        _build_body(ctx, nc, tc, x_d, g_d, be_d, w_d, y_d, cc_in, cc_out)
    nc.compile()
    _CACHE["nc"] = nc
    return nc


def kernel(x, gamma, beta, W):
    x = np.ascontiguousarray(np.asarray(x, dtype=np.float32))
    gamma = np.ascontiguousarray(np.asarray(gamma, dtype=np.float32)).reshape(C, 1)
    beta = np.ascontiguousarray(np.asarray(beta, dtype=np.float32)).reshape(C, 1)
    W = np.ascontiguousarray(np.asarray(W, dtype=np.float32))
    nc = _build()
    in_maps = [
        {
            "x": x[NLOC * k : NLOC * (k + 1)],
            "gamma": gamma,
            "beta": beta,
            "w": W,
        }
        for k in range(N_CORES)
    ]
    res = run_bass_kernel_spmd(nc, in_maps, core_ids=list(range(N_CORES)))
    # y is [128, NLOC, OB, H, W] per core -> [NLOC, OB*128, H, W]
    return np.concatenate(
        [
            np.ascontiguousarray(
                np.transpose(res.results[k]["y"], (1, 2, 0, 3, 4))
            ).reshape(NLOC, C, HW, HW)
            for k in range(N_CORES)
        ],
        axis=0,
    )



# revision 22
# speedup vs baseline: 1.0793x; 1.0793x over previous
"""BinaryConv (BN + sign-binarize + 3x3 binary conv) on 8 Trainium2 NeuronCores.

Strategy (data-parallel over batch, per sharding hint):
  - Each of the 8 cores gets 4 of the 32 images; weights/gamma/beta replicated.
  - Per-core BN partial stats (mean, mean-square per channel) via bn_stats,
    exchanged across cores with a custom 3-round XOR AllGather-hypercube
    built on remote_dma_broadcast (direct SBUF->SBUF peer DMAs + semaphore
    waits inside a tile_critical): round r sends slots [0,2^r) to peer
    (me^2^r), landing at its slots [2^r,2^(r+1)), so slot s always holds
    core (me^s)'s payload and addressing is compile-time uniform SPMD.
    Then one 8-slot DVE reduce: sync-BN exact. This replaces the ncfw CC
    AllGather, whose one-shot cold cost measured ~55us trigger-to-data
    (rendezvous + staged triggers + mesh) vs ~10-25us for the hypercube.
    A dummy CC AllGather (result unused, pinned after the exchange so
    Tile cannot hoist it ahead of the critical section's entry clock)
    stays in the NEFF purely to get the runtime's CC gang-launch
    rendezvous — without any collective the 8 executions stagger by ~ms,
    which the remote_dma waits would eat as dead time.
  - Binarize via ScalarE Sign(gamma*x + (beta*std - mean*gamma)) — equivalent
    to the reference BN+sign since std>0 — into a zero-padded per-image
    layout (33-wide rows with one SHARED pad column per row, both ci-halves
    stacked) in fp8 e4m3.
  - 3x3 conv = 9 shifted DoubleRow fp8 matmuls (contraction 256 in one pass)
    accumulated in PSUM. +/-1 inputs with fp32 PSUM accumulation are exact
    (integer sums), so the conv matches the fp32 reference bit-for-bit.
  - Weights: sign() on ScalarE, transposed to [ci][2][o] via PE transpose.
  - Output is stored partition-major ([128][img][oblk][h][w]) so each
    per-chunk store is one contiguous 1408B burst per partition instead of
    11 separate 128B rows; the host un-permutes after the gather.
  - Post-collective path (readback -> stats math -> first binarize slab ->
    first matmul) is latency-tuned: single strided rank-reduction, fused
    variance algebra, 12-row first binarize slab, and the final image's
    PSUM copies / output DMAs split across DVE/ACT and Sync/ACT queues.
"""

import numpy as np

import concourse.tile as tile
from concourse import bacc, mybir
from concourse.bass_utils import run_bass_kernel_spmd
from concourse.masks import make_identity

F32 = mybir.dt.float32
BF16 = mybir.dt.bfloat16
FP8 = mybir.dt.float8e4

N_CORES = 8
N = 32            # full batch
NLOC = N // N_CORES  # images per core
C = 256           # channels (in == out)
HW = 32           # spatial
CB = C // 128     # ci partition blocks
OB = C // 128     # o partition blocks
EPS = 1e-5

PADW = HW + 1     # padded row width: ONE shared pad column per row — the
                  # zero at 33r+32 serves as right-pad of row r AND left-pad
                  # of row r+1, since both reads want zero
IMG_PAD = 1128    # 34 top zeros + 32*33 interior + 33+ bottom zeros, mult of 8
INT0 = 34         # interior start offset (after the 34-zero top pad)
# output row-chunks (r0, r1): each chunk's matmul free dim = (r1-r0)*33 <= 512
CHUNKS = [(0, 11), (11, 22), (22, 32)]
TAPS = [(dy, dx) for dy in range(3) for dx in range(3)]


def _build_body(ctx, nc, tc, x_d, g_d, be_d, w_d, y_d, cc_in, cc_out):
    # ---------------- pools ----------------
    const = ctx.enter_context(tc.tile_pool(name="const", bufs=1))
    xin_p = ctx.enter_context(tc.tile_pool(name="xin", bufs=1))
    wpool = ctx.enter_context(tc.tile_pool(name="wpool", bufs=1))
    apool = ctx.enter_context(tc.tile_pool(name="apool", bufs=1))
    stat_p = ctx.enter_context(tc.tile_pool(name="stat", bufs=1))
    out_p = ctx.enter_context(tc.tile_pool(name="outp", bufs=1))
    ps_tr = ctx.enter_context(tc.tile_pool(name="pstr", bufs=2, space="PSUM"))
    ps_acc = ctx.enter_context(tc.tile_pool(name="psacc", bufs=1, space="PSUM"))

    # ---------------- load x (stats-critical) ----------------
    # Alternate the two HWDGE issue engines (Sync, Scalar) to halve the
    # descriptor-generation serialization on the input path.
    xin = []
    x_dmas = []
    for b in range(CB):
        xb = xin_p.tile([128, NLOC, HW, HW], F32, name=f"xin{b}", tag=f"xin{b}")
        for i in range(NLOC):
            eng = nc.sync if (b * NLOC + i) % 2 == 0 else nc.scalar
            x_dmas.append(
                eng.dma_start(
                    out=xb[:, i], in_=x_d[i, 128 * b : 128 * (b + 1), :, :]
                )
            )
        xin.append(xb)

    # ---------------- weight prep (independent of stats) ----------------
    ident = const.tile([128, 128], BF16, name="ident")
    make_identity(nc, ident[:])


    # ---------------- zero only the padding of the activation buffers ------
    # (interior is fully overwritten by binarize; tiny strided memsets keep
    # both DVE and the collective-trigger path free)
    apad = [None] * NLOC
    for i in range(NLOC):
        ap = apool.tile([128, CB, IMG_PAD], FP8, name=f"apad{i}",
                        tag=f"apad{i}")
        nc.gpsimd.memset(ap[:, :, 0:INT0], 0.0)
        # shared pad column of each row r sits at INT0 + 33r + 32
        gaps = ap[:, :, INT0 + HW : INT0 + HW + PADW * HW].rearrange(
            "p b (h w) -> p b h w", w=PADW
        )[:, :, :, 0:1]
        nc.gpsimd.memset(gaps, 0.0)
        nc.gpsimd.memset(ap[:, :, INT0 + PADW * HW : IMG_PAD], 0.0)
        apad[i] = ap

    # warm-up rhs: 260 bf16 cols of ones (content irrelevant; the warm-up
    # chain's timing is set by an explicit dep on the stats-readback DMA)
    junk2 = stat_p.tile([128, 260], BF16, name="junk2")
    nc.gpsimd.memset(junk2[:], 1.0)

    # ---------------- local BN stats ----------------
    stats_rec = []
    for b in range(CB):
        xb = xin[b]
        rec = stat_p.tile([128, 2 * NLOC, 6], F32, name=f"rec{b}", tag=f"rec{b}")
        for i in range(NLOC):
            for h in range(2):
                nc.vector.bn_stats(
                    out=rec[:, 2 * i + h, :],
                    in_=xb[:, i, 16 * h : 16 * (h + 1), :].rearrange(
                        "p h w -> p (h w)"
                    ),
                )
        stats_rec.append(rec)

    # pack [mean_b, meansq_b] per ci-block into slot 0 of the gather buffer
    xgat = stat_p.tile([128, N_CORES, 2 * CB], F32, name="xgat", tag="xgat")
    tmp1 = stat_p.tile([128, 1], F32, name="tmp1")
    for b in range(CB):
        mv = stat_p.tile([128, 2], F32, name=f"mv{b}", tag=f"mv{b}")
        nc.vector.bn_aggr(out=mv[:], in_=stats_rec[b][:])
        nc.vector.tensor_copy(
            out=xgat[:, 0, 2 * b : 2 * b + 1], in_=mv[:, 0:1]
        )
        nc.vector.tensor_mul(tmp1[:], mv[:, 0:1], mv[:, 0:1])
        nc.vector.tensor_add(
            xgat[:, 0, 2 * b + 1 : 2 * b + 2], mv[:, 1:2], tmp1[:]
        )

    # ------ stats exchange: 3-round XOR AllGather-hypercube (remote_dma) ----
    # Replaces the ncfw CC AllGather (~55us trigger-to-data measured cold:
    # rendezvous + staged triggers + mesh) with direct SBUF->SBUF peer DMAs:
    # round r sends my slots [0, 2^r) to peer (me ^ 2^r), landing at its
    # slots [2^r, 2^(r+1)); slot s always holds core (me^s)'s payload, so
    # addressing is compile-time uniform SPMD. No adds inside the section
    # (gpsimd ucode ping-pong costs ~5us/switch); one 8-slot DVE reduce in
    # Tile-land afterwards, ordered by the section's exit barrier.
    xrecv = [nc.alloc_semaphore(f"xa_recv{r}") for r in range(3)]
    xlsem = nc.alloc_semaphore("xa_lsem")
    xpsem = nc.alloc_semaphore("xa_psem")
    gp = nc.gpsimd
    with tc.tile_critical(name="xchg"):
        for r in range(3):
            w = 1 << r
            rdests = [None] * 8
            rdests[w] = (0, w)
            gp.remote_dma_broadcast(
                out_ap=xgat[:, w : 2 * w, :],
                in_ap=xgat[:, 0:w, :],
                remote_sem=xrecv[r],
                local_sem=xlsem,
                rdests=rdests,
            ).then_inc(xpsem, 1)
        # Pool enters the section bare and runs the descgens during the
        # stats computation; this marker carries the data-dep wait (slot 0
        # packed) gating the triggers below.
        tc.wait_critical_data_deps()
        gp.wait_ge(xpsem, 3)
        for r in range(3):
            gp.trigger_dma(count=1)
            gp.wait_ge(xrecv[r], 2)
        # no end-of-exec sem clears: the runtime zeroes semaphores between
        # executions (verified: every rep re-waits), and a RANGE_CLEAR with
        # in-flight SDMA references wedges the device.
    # ------- weight prep: gated on the last x DMA so the 2.25MB W transfer
    # never queues ahead of the stats-critical x loads.
    wsign = []
    for o in range(OB):
        wraw = wpool.tile([128, C, 3, 3], F32, name=f"wraw{o}", tag=f"wraw{o}")
        # Gated on x_dmas[-3] completion: overlaps W's 6.4us with the x
        # tail so W lands ~30-33us with LOW cross-core variance — the
        # critical section's entry clock sweeps W's completion in on every
        # core, so the slowest core's W anchors the whole exchange chain.
        # Both alternatives measured worse: ungated or issue-order-only
        # (sync=False) lets W contend with x at the SDMA level (112-204us,
        # huge variance); gating on x_dmas[-1] pushes W to ~35us (104-130us).
        wdma = (nc.sync if o == 0 else nc.scalar).dma_start(
            out=wraw[:], in_=w_d[128 * o : 128 * (o + 1), :, :, :]
        )
        tile.add_dep_helper(
            wdma.ins, x_dmas[-3].ins, sync=True, reason="W after most x loads"
        )
        ws = wpool.tile([128, C, 3, 3], BF16, name=f"wsign{o}", tag=f"wsign{o}")
        wsig_i = nc.scalar.activation(
            out=ws[:], in_=wraw[:], func=mybir.ActivationFunctionType.Sign
        )
        wsign.append(ws)

    # preload the Sqrt activation table while ScalarE is otherwise idle:
    # the real Sqrt right after the stats exchange then skips the ~1.3us
    # ACT_TABLE_LOAD that used to sit on the post-collective critical path
    sqwarm = stat_p.tile([128, 1], F32, name="sqwarm")
    nc.vector.memset(sqwarm[:], 1.0)
    sq_i = nc.scalar.activation(
        out=sqwarm[:], in_=sqwarm[:], func=mybir.ActivationFunctionType.Sqrt
    )
    tile.add_dep_helper(
        sq_i.ins, wsig_i.ins, sync=True, reason="sqrt table after W signs"
    )

    # transposed binarized weights, DoubleRow layout:
    # wT[ci_local, tap*OB+o, ci_half, o_local]  (fp8)
    wT = wpool.tile([128, len(TAPS) * OB, CB, 128], FP8, name="wT")
    for t, (dy, dx) in enumerate(TAPS):
        for b in range(CB):
            for o in range(OB):
                ptr = ps_tr.tile([128, 128], BF16, name="ptr", tag="ptr", bufs=2)
                nc.tensor.transpose(
                    ptr[:], wsign[o][:, 128 * b : 128 * (b + 1), dy, dx], ident[:]
                )
                nc.vector.tensor_copy(
                    out=wT[:, t * OB + o, b, :], in_=ptr[:]
                )

    # reduce the 8 gathered per-core payloads into global sums (ordered
    # after the exchange by the critical section's exit barrier)
    gs = stat_p.tile([128, 2 * CB], F32, name="gs")
    nc.vector.tensor_reduce(
        gs[:],
        xgat[:].rearrange("p k s -> p s k"),
        axis=mybir.AxisListType.X,
        op=mybir.AluOpType.add,
    )
    smean = gs[:].rearrange("p (b s) -> p b s", s=2)[:, :, 0]  # [128, CB]
    smsq = gs[:].rearrange("p (b s) -> p b s", s=2)[:, :, 1]

    # per-channel scale/shift computed as wide [128, CB] ops.
    # Since std > 0:  sign((x-mean)*gamma/std + beta)
    #              == sign(gamma*x + (beta*std - mean*gamma))
    # so scale = gamma (known before the AllReduce!) and
    # shift = beta*std - mean*gamma  (no reciprocal needed).
    eps_t = const.tile([128, 1], F32, name="eps_t")
    nc.vector.memset(eps_t[:], EPS)
    gam = stat_p.tile([128, CB], F32, name="gam")
    bet = stat_p.tile([128, CB], F32, name="bet")
    for b in range(CB):
        nc.sync.dma_start(out=gam[:, b : b + 1], in_=g_d[128 * b : 128 * (b + 1), :])
        nc.sync.dma_start(out=bet[:, b : b + 1], in_=be_d[128 * b : 128 * (b + 1), :])
    inv = 1.0 / N_CORES
    # msqr = inv^2 * smean^2 in one fused op; var = inv*smsq - msqr
    msqr = stat_p.tile([128, CB], F32, name="msqr")
    msqr_i = nc.vector.scalar_tensor_tensor(
        out=msqr[:],
        in0=smean,
        scalar=inv * inv,
        in1=smean,
        op0=mybir.AluOpType.mult,
        op1=mybir.AluOpType.mult,
    )
    var_t = stat_p.tile([128, CB], F32, name="var_t")
    nc.vector.scalar_tensor_tensor(
        out=var_t[:],
        in0=smsq,
        scalar=inv,
        in1=msqr[:],
        op0=mybir.AluOpType.mult,
        op1=mybir.AluOpType.subtract,
    )
    # neg_mg = -(mean)*gamma, computed on DVE in parallel with var
    neg_mg = stat_p.tile([128, CB], F32, name="neg_mg")
    nc.vector.scalar_tensor_tensor(
        out=neg_mg[:],
        in0=smean,
        scalar=-inv,
        in1=gam[:],
        op0=mybir.AluOpType.mult,
        op1=mybir.AluOpType.mult,
    )
    # sqrt on ScalarE; shift = beta*std + neg_mg as two wide DVE ops (DVE is
    # idle here; keeps the ScalarE queue clear for the binarize that follows)
    std_t = stat_p.tile([128, CB], F32, name="std_t")
    nc.scalar.activation(
        out=std_t[:],
        in_=var_t[:],
        func=mybir.ActivationFunctionType.Sqrt,
        bias=eps_t[:],
        scale=1.0,
    )
    sh_t = stat_p.tile([128, CB], F32, name="sh_t")
    nc.vector.tensor_mul(sh_t[:], std_t[:], bet[:])
    nc.vector.tensor_add(sh_t[:], sh_t[:], neg_mg[:])
    scale_t = [gam[:, b : b + 1] for b in range(CB)]
    shift_t = [sh_t[:, b : b + 1] for b in range(CB)]

    # warm-up matmuls (results discarded) — sustain ~100% PE duty through the
    # stat-math + binarize window so the HAM clock ramp COMPLETES before the
    # conv burst. Tiny 4-col warm-ups are dispatch-bound (~4% duty) and leave
    # the clock at mid state: the first ~9 conv matmuls then run at ~313ns
    # instead of ~155ns (measured). 13 x 260-col bf16 matmuls cover the
    # ~3.5us window at full duty and end right as the binarize slabs land.
    for k in range(14):
        pw = ps_tr.tile([128, 260], F32, name="pw", tag="ptr", bufs=2)
        wu = nc.tensor.matmul(pw[:], ident[:], junk2[:], start=True, stop=True)
        if k == 0:
            # start the ramp chain as soon as the exchanged sums land (msqr
            # is the first post-exchange consumer); the 14 matmuls span the
            # stat-math + first-binarize window, ending right at conv start
            tile.add_dep_helper(
                wu.ins, msqr_i.ins, sync=True, reason="warmups with stat math"
            )

    # ---------------- binarize into padded layout (fp8, DoubleRow pairs) ----
    # Row-halves so the first conv chunk (rows 0..13) can start as soon as
    # the top halves of both ci-blocks are written.
    # first slab is just the 12 rows conv chunk 0 needs, so the first
    # matmul starts ~0.9us sooner after the stats land
    BROWS = [(0, 12), (12, 32)]
    for i in range(NLOC):
        for (hr0, hr1) in BROWS:
            for b in range(CB):
                interior = apad[i][:, b, INT0 : INT0 + PADW * HW].rearrange(
                    "p (h w) -> p h w", w=PADW
                )[:, hr0:hr1, 0:HW]
                nc.scalar.activation(
                    out=interior,
                    in_=xin[b][:, i, hr0:hr1, :],
                    func=mybir.ActivationFunctionType.Sign,
                    scale=scale_t[b],
                    bias=shift_t[b],
                )

    # ---------------- conv: 9 shifted DoubleRow matmuls, PSUM accumulate ----
    # Chunk-outer / taps-inner: each row-chunk's accumulation closes ~3-7us
    # before the image's last matmul, so its PSUM copy + output store drain
    # DURING compute — the final image no longer flushes 1MB after the last
    # matmul, only its last chunk.
    for i in range(NLOC):
        osb = {}
        for o in range(OB):
            osb[o] = out_p.tile([128, HW, HW], F32, name=f"osb{o}",
                                tag=f"osb{o}", bufs=2)
        for ci, (r0, r1) in enumerate(CHUNKS):
            ncols = (r1 - r0) * PADW
            psum = {}
            for o in range(OB):
                psum[o] = ps_acc.tile(
                    [128, ncols], F32, name=f"acc{o}_{ci}",
                    tag=f"acc{o}_{ci}", bufs=1,
                )
            for t, (dy, dx) in enumerate(TAPS):
                # interior(r0+dy-1, dx-1) = INT0 + 33*(r0+dy-1) + dx-1
                off = PADW * (r0 + dy) + dx
                for o in range(OB):
                    nc.tensor.matmul(
                        psum[o][:],
                        wT[:, t * OB + o, :, :],
                        apad[i][:, :, off : off + ncols],
                        start=t == 0,
                        stop=t == len(TAPS) - 1,
                        perf_mode=mybir.MatmulPerfMode.DoubleRow,
                    )
            for o in range(OB):
                src = psum[o][:].rearrange("p (r c) -> p r c", c=PADW)[
                    :, :, 0:HW
                ]
                # o=1 copy+store ride the ACT queue once binarize is done
                # (i>=2), halving the per-chunk serial copy+issue chain;
                # the output DMA issues from the same engine as its copy.
                if i >= 2 and o == 1:
                    nc.scalar.copy(out=osb[o][:, r0:r1, :], in_=src)
                    deng = nc.scalar
                else:
                    nc.vector.tensor_copy(out=osb[o][:, r0:r1, :], in_=src)
                    deng = nc.sync
                deng.dma_start(
                    out=y_d[:, i, o, r0:r1, :],
                    in_=osb[o][:, r0:r1, :],
                )

    # ---------------- dummy collective (gang-launch rendezvous) ------------
    # A CC-NEFF makes the runtime co-schedule the 8 per-core executions
    # (cores start within ~20us; without any collective they stagger by ~ms,
    # which the remote_dma exchange would eat as dead wait time). Emitted
    # LAST so nothing gates on its ~40us cold completion — it was measured
    # to stall the critical section until mesh-end when emitted first. The
    # gathered result is never consumed; the mesh runs on the TOPSP engines
    # concurrently with the conv tail / kernel drain.
    ccdummy = stat_p.tile([128, 2 * CB], F32, name="ccdummy")
    nc.vector.memset(ccdummy[:], 0.0)
    ccb = nc.gpsimd.dma_start(out=cc_in[:, :], in_=ccdummy[:])
    cc_i = nc.gpsimd.collective_compute(
        "AllGather",
        mybir.AluOpType.bypass,
        replica_groups=[list(range(N_CORES))],
        ins=[cc_in.ap().opt()],
        outs=[cc_out.ap().opt()],
    )
    # Pin the dummy CC behind the first post-exchange instruction: left
    # unpinned, Tile hoists it to the front of gpsimd's schedule and the
    # critical section's entry clock then waits for the CC mesh (~90us).
    # Scheduled here its mesh overlaps the conv and finishes before drain.
    tile.add_dep_helper(ccb.ins, msqr_i.ins, sync=True, reason="CC after stats")
    tile.add_dep_helper(cc_i.ins, msqr_i.ins, sync=True, reason="CC after stats")


_CACHE: dict = {}


def _build():
    if "nc" in _CACHE:
        return _CACHE["nc"]
    nc = bacc.Bacc(
        "TRN2", target_bir_lowering=False, debug=False, num_devices=N_CORES
    )
    x_d = nc.dram_tensor("x", [NLOC, C, HW, HW], F32, kind="ExternalInput")
    g_d = nc.dram_tensor("gamma", [C, 1], F32, kind="ExternalInput")
    be_d = nc.dram_tensor("beta", [C, 1], F32, kind="ExternalInput")
    w_d = nc.dram_tensor("w", [C, C, 3, 3], F32, kind="ExternalInput")
    # partition-major output layout: per-partition contiguous 1408B DMA
    # writes; the host un-permutes to [NLOC, C, H, W] for free
    y_d = nc.dram_tensor("y", [128, NLOC, OB, HW, HW], F32, kind="ExternalOutput")
    cc_in = nc.dram_tensor("cc_in", [128, 2 * CB], F32)
    cc_out = nc.dram_tensor(
        "cc_out", [N_CORES, 128, 2 * CB], F32, addr_space="Shared"
    )

    from contextlib import ExitStack

    with tile.TileContext(nc) as tc, ExitStack() as ctx:
        _build_body(ctx, nc, tc, x_d, g_d, be_d, w_d, y_d, cc_in, cc_out)
    nc.compile()
    _CACHE["nc"] = nc
    return nc


def kernel(x, gamma, beta, W):
    x = np.ascontiguousarray(np.asarray(x, dtype=np.float32))
    gamma = np.ascontiguousarray(np.asarray(gamma, dtype=np.float32)).reshape(C, 1)
    beta = np.ascontiguousarray(np.asarray(beta, dtype=np.float32)).reshape(C, 1)
    W = np.ascontiguousarray(np.asarray(W, dtype=np.float32))
    nc = _build()
    in_maps = [
        {
            "x": x[NLOC * k : NLOC * (k + 1)],
            "gamma": gamma,
            "beta": beta,
            "w": W,
        }
        for k in range(N_CORES)
    ]
    res = run_bass_kernel_spmd(nc, in_maps, core_ids=list(range(N_CORES)))
    # y is [128, NLOC, OB, H, W] per core -> [NLOC, OB*128, H, W]
    return np.concatenate(
        [
            np.ascontiguousarray(
                np.transpose(res.results[k]["y"], (1, 2, 0, 3, 4))
            ).reshape(NLOC, C, HW, HW)
            for k in range(N_CORES)
        ],
        axis=0,
    )

